# revision 10
# baseline (speedup 1.0000x reference)
"""Distributed 2-layer GCN (PyG GCNConv-style) on 8 Trainium2 NeuronCores.

Strategy (matches the sharding hint):
  - Nodes are sharded 2500/core (padded to 2560 = 20 blocks of 128).
  - Edges are partitioned by destination node; per (core, dst-block) the
    incoming edges (+ self loops) are packed into uniform KT tiles of 128
    edge slots. The sym-normalization coefficients are precomputed on the
    host into small per-block selection matrices S [128 edge-slots, 128 dst]
    so that scatter-add becomes a TensorE matmul.
  - Layer 1 aggregates in INPUT space (A @ x, width F=512) before the W1
    matmul; layer 2 aggregates AFTER the W2 projection (A @ (h@W2), width
    128). Both choices minimize gathered bytes.
  - The full (padded) x table is replicated to every core's HBM so layer-1
    gathers are local. Layer 2 needs remote h@W2 rows -> one AllGather of
    the [20480, 128] hw table (1.3 MB/core on the wire).
  - Weights are replicated; everything is laid out feature-major on the
    host so the kernel needs zero on-device transposes.

kernel(**inputs) takes the FULL unsharded inputs and returns the FULL
[20000, 128] float32 output.
"""

import math

import numpy as np
import ml_dtypes

import concourse.bass as bass
import concourse.mybir as mybir
import concourse.tile as tile
from concourse import bacc
from concourse.bass_utils import run_bass_kernel_spmd

# ----------------------------------------------------------------------------
# configuration
# ----------------------------------------------------------------------------
C = 8  # cores

# compute dtype for gathers / matmul operands ("bf16" or "f32").
# PSUM accumulation is always fp32.
COMPUTE = "f32"

_DT = {
    "bf16": (mybir.dt.bfloat16, ml_dtypes.bfloat16),
    "f32": (mybir.dt.float32, np.float32),
}

_prog_cache: dict = {}


def _cfg_from_shapes(x, w1, w2):
    n, f = x.shape
    h = w1.shape[1]
    out = w2.shape[1]
    assert n % C == 0, n
    nl = n // C                      # real nodes per core
    nlp = ((nl + 127) // 128) * 128  # padded nodes per core
    b = nlp // 128                   # dst blocks per core
    assert f % 128 == 0 and h % 128 == 0 and out % 128 == 0
    return dict(N=n, F=f, H=h, OUT=out, NL=nl, NLP=nlp, B=b, NP=C * nlp,
                FK=f // 128, HC=h // 128, OC=out // 128)


# ----------------------------------------------------------------------------
# host-side preprocessing: graph partition + norm coefficients + layouts
# ----------------------------------------------------------------------------
def _preprocess(x, edge_index, edge_weight, w1, b1, w2, b2, rw1, rb1, rw2, rb2,
                cfg, np_cdt):
    N, F, H, OUT = cfg["N"], cfg["F"], cfg["H"], cfg["OUT"]
    NL, NLP, B, NP = cfg["NL"], cfg["NLP"], cfg["B"], cfg["NP"]
    HC, FK = cfg["HC"], cfg["FK"]

    row = np.asarray(edge_index[0], dtype=np.int64)
    col = np.asarray(edge_index[1], dtype=np.int64)
    ew = np.asarray(edge_weight, dtype=np.float32)

    # symmetric normalization, exactly like the reference (self loop weight 1)
    deg = np.bincount(col, weights=ew.astype(np.float64), minlength=N) + 1.0
    deg = deg.astype(np.float32)
    dis = np.where(deg > 0, 1.0 / np.sqrt(np.where(deg > 0, deg, 1.0)), 0.0)
    dis = dis.astype(np.float32)

    loop = np.arange(N, dtype=np.int64)
    srcs = np.concatenate([row, loop])
    dsts = np.concatenate([col, loop])
    norms = np.concatenate([dis[row] * ew * dis[col], dis * dis])

    # padded node ids: node g lives on core g//NL at local slot g%NL
    src_pad = (srcs // NL) * NLP + (srcs % NL)
    core = dsts // NL
    local = dsts % NL
    blk = local // 128
    dloc = local % 128

    key = (core * B + blk).astype(np.int64)
    order = np.argsort(key, kind="stable")
    key_s = key[order]
    counts = np.bincount(key_s, minlength=C * B)
    starts = np.zeros(C * B, dtype=np.int64)
    np.cumsum(counts[:-1], out=starts[1:])
    pos = np.arange(key_s.size, dtype=np.int64) - starts[key_s]

    KT = max(1, int(math.ceil(counts.max() / 128)))
    EPB = KT * 128

    src_s = src_pad[order].astype(np.int32)
    core_s = core[order]
    blk_s = blk[order]
    dloc_s = dloc[order]
    norm_s = norms[order]
    kt_s = pos // 128
    p_s = pos % 128

    idx_all = np.zeros((C, 128, B * KT), dtype=np.int32)
    idx_all[core_s, p_s, blk_s * KT + kt_s] = src_s

    S_all = np.zeros((C, B, 128, KT * 128), dtype=np.float32)
    S_all[core_s, blk_s, p_s, kt_s * 128 + dloc_s] = norm_s
    S_all = S_all.astype(np_cdt)

    # padded, replicated x table [NP, F]
    x = np.asarray(x, dtype=np.float32)
    x_table = np.zeros((NP, F), dtype=np.float32)
    x_table.reshape(C, NLP, F)[:, :NL] = x.reshape(C, NL, F)
    x_table = x_table.astype(np_cdt)

    # feature-major x per core: xT[p, k, n] = x_core[n, k*128+p]
    xT_all = np.ascontiguousarray(
        x_table.reshape(C, NLP, FK, 128).transpose(0, 3, 2, 1))

    w1 = np.asarray(w1, np.float32)
    rw1 = np.asarray(rw1, np.float32)
    w2 = np.asarray(w2, np.float32)
    rw2 = np.asarray(rw2, np.float32)
    b1c = (np.asarray(b1, np.float32) + np.asarray(rb1, np.float32))
    b2c = (np.asarray(b2, np.float32) + np.asarray(rb2, np.float32))

    # [128, FK, H] : w1_in[p, k, j] = w1[k*128+p, j]
    w1_in = np.ascontiguousarray(
        w1.reshape(FK, 128, H).transpose(1, 0, 2)).astype(np_cdt)
    rw1_in = np.ascontiguousarray(
        rw1.reshape(FK, 128, H).transpose(1, 0, 2)).astype(np_cdt)
    w2_in = np.ascontiguousarray(
        w2.reshape(HC, 128, OUT).transpose(1, 0, 2)).astype(np_cdt)
    rw2b_in = np.zeros((128, HC + 1, OUT), dtype=np.float32)
    rw2b_in[:, :HC] = rw2.reshape(HC, 128, OUT).transpose(1, 0, 2)
    rw2b_in[0, HC, :] = b2c
    rw2b_in = rw2b_in.astype(np_cdt)

    bias1_in = np.ascontiguousarray(b1c.reshape(HC, 128).T).astype(np.float32)

    in_maps = []
    for c in range(C):
        in_maps.append({
            "x_table": x_table,
            "idx_in": np.ascontiguousarray(idx_all[c]),
            "s_in": np.ascontiguousarray(S_all[c]),
            "xt_in": np.ascontiguousarray(xT_all[c]),
            "w1_in": w1_in,
            "rw1_in": rw1_in,
            "w2_in": w2_in,
            "rw2b_in": rw2b_in,
            "bias1_in": bias1_in,
        })
    return in_maps, KT


# ----------------------------------------------------------------------------
# device program
# ----------------------------------------------------------------------------
def _build(cfg, KT, cdt, gather_mode="per_kt", debug_out=False):
    F, H, OUT = cfg["F"], cfg["H"], cfg["OUT"]
    NLP, B, NP = cfg["NLP"], cfg["B"], cfg["NP"]
    FK, HC, OC = cfg["FK"], cfg["HC"], cfg["OC"]
    f32 = mybir.dt.float32

    nc = bacc.Bacc("TRN2", target_bir_lowering=False, debug=False,
                   enable_asserts=False, num_devices=C)

    x_table = nc.dram_tensor("x_table", [NP, F], cdt, kind="ExternalInput")
    idx_in = nc.dram_tensor("idx_in", [128, B * KT], mybir.dt.int32,
                            kind="ExternalInput")
    s_in = nc.dram_tensor("s_in", [B, 128, KT * 128], cdt,
                          kind="ExternalInput")
    xt_in = nc.dram_tensor("xt_in", [128, FK, NLP], cdt, kind="ExternalInput")
    w1_in = nc.dram_tensor("w1_in", [128, FK, H], cdt, kind="ExternalInput")
    rw1_in = nc.dram_tensor("rw1_in", [128, FK, H], cdt, kind="ExternalInput")
    w2_in = nc.dram_tensor("w2_in", [128, HC, OUT], cdt, kind="ExternalInput")
    rw2b_in = nc.dram_tensor("rw2b_in", [128, HC + 1, OUT], cdt,
                             kind="ExternalInput")
    bias1_in = nc.dram_tensor("bias1_in", [128, HC], f32,
                              kind="ExternalInput")
    out_d = nc.dram_tensor("out", [NLP, OUT], f32, kind="ExternalOutput")
    if debug_out:
        dbg_hwfull = nc.dram_tensor("dbg_hwfull", [NP, OUT], f32,
                                    kind="ExternalOutput")
        dbg_ht = nc.dram_tensor("dbg_ht", [128, HC, NLP], f32,
                                kind="ExternalOutput")
        dbg_ax = nc.dram_tensor("dbg_ax", [128, FK, NLP], f32,
                                kind="ExternalOutput")

    def _gather(out_tile, table, idx_sb, b):
        if gather_mode == "multi":
            nc.gpsimd.indirect_dma_start(
                out=out_tile[:], out_offset=None, in_=table[:],
                in_offset=bass.IndirectOffsetOnAxis(
                    ap=idx_sb[:, b * KT:(b + 1) * KT], axis=0))
        else:
            for kt in range(KT):
                nc.gpsimd.indirect_dma_start(
                    out=out_tile[:, kt], out_offset=None, in_=table[:],
                    in_offset=bass.IndirectOffsetOnAxis(
                        ap=idx_sb[:, b * KT + kt:b * KT + kt + 1], axis=0))

    with tile.TileContext(nc) as tc:
        with (
            tc.tile_pool(name="dram", bufs=1, space="DRAM") as dram,
            tc.tile_pool(name="const", bufs=1) as const,
        ):
            # internal DRAM
            hT_d = dram.tile([128, HC, NLP], cdt)
            hw_loc = dram.tile([NLP, OUT], cdt)
            hw_full = dram.tile([NP, OUT], cdt, addr_space="Shared")

            # resident constants
            w1_sb = const.tile([128, FK, H], cdt)
            nc.sync.dma_start(out=w1_sb[:], in_=w1_in[:])
            rw1_sb = const.tile([128, FK, H], cdt)
            nc.sync.dma_start(out=rw1_sb[:], in_=rw1_in[:])
            w2_sb = const.tile([128, HC, OUT], cdt)
            nc.sync.dma_start(out=w2_sb[:], in_=w2_in[:])
            rw2b_sb = const.tile([128, HC + 1, OUT], cdt)
            nc.sync.dma_start(out=rw2b_sb[:], in_=rw2b_in[:])
            bias1_sb = const.tile([128, HC], f32)
            nc.sync.dma_start(out=bias1_sb[:], in_=bias1_in[:])
            idx_sb = const.tile([128, B * KT], mybir.dt.int32)
            nc.sync.dma_start(out=idx_sb[:], in_=idx_in[:])
            xt_sb = const.tile([128, FK, NLP], cdt)
            nc.sync.dma_start(out=xt_sb[:], in_=xt_in[:])
            ones_sb = const.tile([128, 128], cdt)
            nc.vector.memset(ones_sb[:], 0.0)
            nc.vector.memset(ones_sb[0:1, :], 1.0)

            # ---------------- phase A: layer 1 + hw ----------------
            with (
                tc.tile_pool(name="xg_pool", bufs=2) as xg_pool,
                tc.tile_pool(name="sa_pool", bufs=2) as sa_pool,
                tc.tile_pool(name="axsb_pool", bufs=2) as axsb_pool,
                tc.tile_pool(name="hstage_pool", bufs=3) as hstage_pool,
                tc.tile_pool(name="hwsb_pool", bufs=2) as hwsb_pool,
                tc.tile_pool(name="ax_psum", bufs=2, space="PSUM") as ax_psum,
                tc.tile_pool(name="h_psum", bufs=2, space="PSUM") as h_psum,
                tc.tile_pool(name="hw_psum", bufs=2, space="PSUM") as hw_psum,
            ):
                for b in range(B):
                    bw = slice(b * 128, (b + 1) * 128)
                    s_sb = sa_pool.tile([128, KT * 128], cdt, tag="s_sb")
                    nc.sync.dma_start(out=s_sb[:], in_=s_in[b])
                    xg = xg_pool.tile([128, KT, F], cdt, tag="xg")
                    _gather(xg, x_table, idx_sb, b)

                    # aggregation in input space: axT[fc] = Xg_chunk.T @ S
                    psum_ax = ax_psum.tile([128, FK, 128], f32, tag="psum_ax")
                    for fc in range(FK):
                        for kt in range(KT):
                            nc.tensor.matmul(
                                out=psum_ax[:, fc, :],
                                lhsT=xg[:, kt, fc * 128:(fc + 1) * 128],
                                rhs=s_sb[:, kt * 128:(kt + 1) * 128],
                                start=(kt == 0),
                                stop=(kt == KT - 1),
                            )
                    axT_sb = axsb_pool.tile([128, FK, 128], cdt, tag="axT_sb")
                    nc.vector.tensor_copy(out=axT_sb[:], in_=psum_ax[:])
                    if debug_out:
                        nc.gpsimd.dma_start(out=dbg_ax[:, :, bw],
                                            in_=axT_sb[:])

                    # dense: hT = relu(W1.T @ axT + RW1.T @ xT + b1c)
                    h_tiles = []
                    for half in range(2):
                        psum_h = h_psum.tile([128, HC // 2, 128], f32,
                                             tag="psum_h")
                        for j in range(HC // 2):
                            hc = half * (HC // 2) + j
                            hs = slice(hc * 128, (hc + 1) * 128)
                            for k in range(FK):
                                nc.tensor.matmul(
                                    out=psum_h[:, j, :],
                                    lhsT=w1_sb[:, k, hs],
                                    rhs=axT_sb[:, k, :],
                                    start=(k == 0), stop=False)
                            for k in range(FK):
                                nc.tensor.matmul(
                                    out=psum_h[:, j, :],
                                    lhsT=rw1_sb[:, k, hs],
                                    rhs=xt_sb[:, k, bw],
                                    start=False, stop=(k == FK - 1))
                        hT_sb = hstage_pool.tile([128, HC // 2, 128], cdt,
                                                 tag="hT_sb")
                        for j in range(HC // 2):
                            hc = half * (HC // 2) + j
                            nc.scalar.activation(
                                out=hT_sb[:, j, :], in_=psum_h[:, j, :],
                                func=mybir.ActivationFunctionType.Relu,
                                bias=bias1_sb[:, hc:hc + 1], scale=1.0)
                        nc.sync.dma_start(
                            out=hT_d[:, half * (HC // 2):(half + 1) * (HC // 2), bw],
                            in_=hT_sb[:])
                        if debug_out:
                            nc.gpsimd.dma_start(
                                out=dbg_ht[:, half * (HC // 2):(half + 1) * (HC // 2), bw],
                                in_=hT_sb[:])
                        h_tiles.append(hT_sb)

                    # hw = h @ W2   (node-major)
                    psum_hw = hw_psum.tile([128, OC, 128], f32, tag="psum_hw")
                    for oc in range(OC):
                        for half in range(2):
                            for j in range(HC // 2):
                                hc = half * (HC // 2) + j
                                nc.tensor.matmul(
                                    out=psum_hw[:, oc, :],
                                    lhsT=h_tiles[half][:, j, :],
                                    rhs=w2_sb[:, hc, oc * 128:(oc + 1) * 128],
                                    start=(half == 0 and j == 0),
                                    stop=(half == 1 and j == HC // 2 - 1))
                    hw_sb = hwsb_pool.tile([128, OUT], cdt, tag="hw_sb")
                    nc.vector.tensor_copy(out=hw_sb[:], in_=psum_hw[:])
                    nc.sync.dma_start(out=hw_loc[bw, :], in_=hw_sb[:])

            # ---------------- all-gather of hw ----------------
            nc.gpsimd.collective_compute(
                "AllGather",
                mybir.AluOpType.bypass,
                replica_groups=[list(range(C))],
                ins=[hw_loc[:].opt()],
                outs=[hw_full[:].opt()],
            )
            if debug_out:
                nc.gpsimd.dma_start(out=dbg_hwfull[:], in_=hw_full[:])

            # ---------------- phase B: layer 2 ----------------
            with (
                tc.tile_pool(name="sb_pool", bufs=2) as sb_pool,
                tc.tile_pool(name="hwg_pool", bufs=2) as hwg_pool,
                tc.tile_pool(name="hblk_pool", bufs=2) as hblk_pool,
                tc.tile_pool(name="osb_pool", bufs=2) as osb_pool,
                tc.tile_pool(name="o_psum", bufs=2, space="PSUM") as o_psum,
            ):
                for b in range(B):
                    bw = slice(b * 128, (b + 1) * 128)
                    s_sb2 = sb_pool.tile([128, KT * 128], cdt, tag="s_sb2")
                    nc.sync.dma_start(out=s_sb2[:], in_=s_in[b])
                    hwg = hwg_pool.tile([128, KT, OUT], cdt, tag="hwg")
                    _gather(hwg, hw_full, idx_sb, b)
                    hblk = hblk_pool.tile([128, HC, 128], cdt, tag="hblk")
                    nc.sync.dma_start(out=hblk[:], in_=hT_d[:, :, bw])

                    psum_o = o_psum.tile([128, OC, 128], f32, tag="psum_o")
                    for oc in range(OC):
                        ow = slice(oc * 128, (oc + 1) * 128)
                        for kt in range(KT):
                            nc.tensor.matmul(
                                out=psum_o[:, oc, :],
                                lhsT=s_sb2[:, kt * 128:(kt + 1) * 128],
                                rhs=hwg[:, kt, ow],
                                start=(kt == 0), stop=False)
                        for k in range(HC):
                            nc.tensor.matmul(
                                out=psum_o[:, oc, :],
                                lhsT=hblk[:, k, :],
                                rhs=rw2b_sb[:, k, ow],
                                start=False, stop=False)
                        nc.tensor.matmul(
                            out=psum_o[:, oc, :],
                            lhsT=ones_sb[:],
                            rhs=rw2b_sb[:, HC, ow],
                            start=False, stop=True)
                    out_sb = osb_pool.tile([128, OUT], f32, tag="out_sb")
                    nc.vector.tensor_copy(out=out_sb[:], in_=psum_o[:])
                    nc.sync.dma_start(out=out_d[bw, :], in_=out_sb[:])

    nc.compile()
    return nc


# ----------------------------------------------------------------------------
# entry points
# ----------------------------------------------------------------------------
def _run(inputs, trace=False, compute=None, trace_kwargs=None):
    compute = compute or COMPUTE
    cdt, np_cdt = _DT[compute]
    x = np.asarray(inputs["x"])
    cfg = _cfg_from_shapes(x, np.asarray(inputs["w1"]),
                           np.asarray(inputs["w2"]))
    in_maps, KT = _preprocess(
        x, inputs["edge_index"], inputs["edge_weight"],
        inputs["w1"], inputs["b1"], inputs["w2"], inputs["b2"],
        inputs["rw1"], inputs["rb1"], inputs["rw2"], inputs["rb2"],
        cfg, np_cdt)

    key = (tuple(sorted(cfg.items())), KT, compute)
    nc = _prog_cache.get(key)
    if nc is None:
        nc = _build(cfg, KT, cdt)
        _prog_cache[key] = nc

    res = run_bass_kernel_spmd(
        nc, in_maps, core_ids=list(range(C)), trace=trace,
        **(trace_kwargs or {}))

    NL, NLP = cfg["NL"], cfg["NLP"]
    out = np.concatenate(
        [res.results[c]["out"][:NL] for c in range(C)], axis=0)
    return np.ascontiguousarray(out.astype(np.float32)), res


def kernel(**inputs) -> np.ndarray:
    out, _ = _run(inputs, trace=False)
    return out


# revision 15
# speedup vs baseline: 140.2615x; 140.2615x over previous
"""Distributed 2-layer GCN (PyG GCNConv-style) on 8 Trainium2 NeuronCores.

Strategy (matches the sharding hint):
  - Nodes are sharded 2500/core (padded to 2560 = 20 blocks of 128).
  - Edges are partitioned by destination node; per (core, dst-block) the
    incoming edges (+ self loops) are packed into uniform KT tiles of 128
    edge slots. The sym-normalization coefficients are precomputed on the
    host into small per-block selection matrices S [128 edge-slots, 128 dst]
    so that scatter-add becomes a TensorE matmul.
  - Layer 1 aggregates in INPUT space (A @ x, width F=512) before the W1
    matmul; layer 2 aggregates AFTER the W2 projection (A @ (h@W2), width
    128). Both choices minimize gathered bytes.
  - The full (padded) x table is replicated to every core's HBM so layer-1
    gathers are local. Layer 2 needs remote h@W2 rows -> one AllGather of
    the [20480, 128] hw table (1.3 MB/core on the wire).
  - Weights are replicated; everything is laid out feature-major on the
    host so the kernel needs zero on-device transposes.

kernel(**inputs) takes the FULL unsharded inputs and returns the FULL
[20000, 128] float32 output.
"""

import math

import numpy as np
import ml_dtypes

import concourse.bass as bass
import concourse.mybir as mybir
import concourse.tile as tile
from concourse import bacc
from concourse.bass_utils import run_bass_kernel_spmd

# ----------------------------------------------------------------------------
# configuration
# ----------------------------------------------------------------------------
C = 8  # cores

# compute dtype for gathers / matmul operands ("bf16" or "f32").
# PSUM accumulation is always fp32. bf16 measured 3.2e-3 rel err vs the
# fp32 reference and roughly halves both HBM traffic and PE time.
COMPUTE = "bf16"

_DT = {
    "bf16": (mybir.dt.bfloat16, ml_dtypes.bfloat16),
    "f32": (mybir.dt.float32, np.float32),
}

_prog_cache: dict = {}


def _cfg_from_shapes(x, w1, w2):
    n, f = x.shape
    h = w1.shape[1]
    out = w2.shape[1]
    assert n % C == 0, n
    nl = n // C                      # real nodes per core
    nlp = ((nl + 127) // 128) * 128  # padded nodes per core
    b = nlp // 128                   # dst blocks per core
    assert f % 128 == 0 and h % 128 == 0 and out % 128 == 0
    return dict(N=n, F=f, H=h, OUT=out, NL=nl, NLP=nlp, B=b, NP=C * nlp,
                FK=f // 128, HC=h // 128, OC=out // 128)


# ----------------------------------------------------------------------------
# host-side preprocessing: graph partition + norm coefficients + layouts
# ----------------------------------------------------------------------------
def _preprocess(x, edge_index, edge_weight, w1, b1, w2, b2, rw1, rb1, rw2, rb2,
                cfg, np_cdt):
    N, F, H, OUT = cfg["N"], cfg["F"], cfg["H"], cfg["OUT"]
    NL, NLP, B, NP = cfg["NL"], cfg["NLP"], cfg["B"], cfg["NP"]
    HC, FK = cfg["HC"], cfg["FK"]

    row = np.asarray(edge_index[0], dtype=np.int64)
    col = np.asarray(edge_index[1], dtype=np.int64)
    ew = np.asarray(edge_weight, dtype=np.float32)

    # symmetric normalization, exactly like the reference (self loop weight 1)
    deg = np.bincount(col, weights=ew.astype(np.float64), minlength=N) + 1.0
    deg = deg.astype(np.float32)
    dis = np.where(deg > 0, 1.0 / np.sqrt(np.where(deg > 0, deg, 1.0)), 0.0)
    dis = dis.astype(np.float32)

    loop = np.arange(N, dtype=np.int64)
    srcs = np.concatenate([row, loop])
    dsts = np.concatenate([col, loop])
    norms = np.concatenate([dis[row] * ew * dis[col], dis * dis])

    # padded node ids: node g lives on core g//NL at local slot g%NL
    src_pad = (srcs // NL) * NLP + (srcs % NL)
    core = dsts // NL
    local = dsts % NL
    blk = local // 128
    dloc = local % 128

    key = (core * B + blk).astype(np.int64)
    order = np.argsort(key, kind="stable")
    key_s = key[order]
    counts = np.bincount(key_s, minlength=C * B)
    starts = np.zeros(C * B, dtype=np.int64)
    np.cumsum(counts[:-1], out=starts[1:])
    pos = np.arange(key_s.size, dtype=np.int64) - starts[key_s]

    KT = max(1, int(math.ceil(counts.max() / 128)))

    src_s = src_pad[order].astype(np.int32)
    core_s = core[order]
    blk_s = blk[order]
    dloc_s = dloc[order]
    norm_s = norms[order]
    kt_s = pos // 128
    p_s = pos % 128

    idx_all = np.zeros((C, 128, B * KT), dtype=np.int32)
    idx_all[core_s, p_s, blk_s * KT + kt_s] = src_s

    S_all = np.zeros((C, B, 128, KT * 128), dtype=np.float32)
    S_all[core_s, blk_s, p_s, kt_s * 128 + dloc_s] = norm_s
    S_all = S_all.astype(np_cdt)

    # padded, replicated x table [NP, F]
    x = np.asarray(x, dtype=np.float32)
    x_table = np.zeros((NP, F), dtype=np.float32)
    x_table.reshape(C, NLP, F)[:, :NL] = x.reshape(C, NL, F)
    x_table = x_table.astype(np_cdt)

    # feature-major x per core: xT[p, k, n] = x_core[n, k*128+p]
    xT_all = np.ascontiguousarray(
        x_table.reshape(C, NLP, FK, 128).transpose(0, 3, 2, 1))

    w1 = np.asarray(w1, np.float32)
    rw1 = np.asarray(rw1, np.float32)
    w2 = np.asarray(w2, np.float32)
    rw2 = np.asarray(rw2, np.float32)
    b1c = (np.asarray(b1, np.float32) + np.asarray(rb1, np.float32))
    b2c = (np.asarray(b2, np.float32) + np.asarray(rb2, np.float32))

    # [128, FK, H] : w1_in[p, k, j] = w1[k*128+p, j]
    w1_in = np.ascontiguousarray(
        w1.reshape(FK, 128, H).transpose(1, 0, 2)).astype(np_cdt)
    rw1_in = np.ascontiguousarray(
        rw1.reshape(FK, 128, H).transpose(1, 0, 2)).astype(np_cdt)
    w2_in = np.ascontiguousarray(
        w2.reshape(HC, 128, OUT).transpose(1, 0, 2)).astype(np_cdt)
    rw2b_in = np.zeros((128, HC + 1, OUT), dtype=np.float32)
    rw2b_in[:, :HC] = rw2.reshape(HC, 128, OUT).transpose(1, 0, 2)
    rw2b_in[0, HC, :] = b2c
    rw2b_in = rw2b_in.astype(np_cdt)

    bias1_in = np.ascontiguousarray(b1c.reshape(HC, 128).T).astype(np.float32)

    in_maps = []
    for c in range(C):
        in_maps.append({
            "x_table": x_table,
            "idx_in": np.ascontiguousarray(idx_all[c]),
            "s_in": np.ascontiguousarray(S_all[c]),
            "xt_in": np.ascontiguousarray(xT_all[c]),
            "w1_in": w1_in,
            "rw1_in": rw1_in,
            "w2_in": w2_in,
            "rw2b_in": rw2b_in,
            "bias1_in": bias1_in,
        })
    return in_maps, KT


# ----------------------------------------------------------------------------
# device program
# ----------------------------------------------------------------------------
def _build(cfg, KT, cdt, gather_mode="per_kt", debug_out=False, reps=1,
           no_collective=False):
    F, H, OUT = cfg["F"], cfg["H"], cfg["OUT"]
    NLP, B, NP = cfg["NLP"], cfg["B"], cfg["NP"]
    FK, HC, OC = cfg["FK"], cfg["HC"], cfg["OC"]
    f32 = mybir.dt.float32

    nc = bacc.Bacc("TRN2", target_bir_lowering=False, debug=False,
                   enable_asserts=False, num_devices=C)

    x_table = nc.dram_tensor("x_table", [NP, F], cdt, kind="ExternalInput")
    idx_in = nc.dram_tensor("idx_in", [128, B * KT], mybir.dt.int32,
                            kind="ExternalInput")
    s_in = nc.dram_tensor("s_in", [B, 128, KT * 128], cdt,
                          kind="ExternalInput")
    xt_in = nc.dram_tensor("xt_in", [128, FK, NLP], cdt, kind="ExternalInput")
    w1_in = nc.dram_tensor("w1_in", [128, FK, H], cdt, kind="ExternalInput")
    rw1_in = nc.dram_tensor("rw1_in", [128, FK, H], cdt, kind="ExternalInput")
    w2_in = nc.dram_tensor("w2_in", [128, HC, OUT], cdt, kind="ExternalInput")
    rw2b_in = nc.dram_tensor("rw2b_in", [128, HC + 1, OUT], cdt,
                             kind="ExternalInput")
    bias1_in = nc.dram_tensor("bias1_in", [128, HC], f32,
                              kind="ExternalInput")
    out_d = nc.dram_tensor("out", [NLP, OUT], f32, kind="ExternalOutput")
    if debug_out:
        dbg_hwfull = nc.dram_tensor("dbg_hwfull", [NP, OUT], f32,
                                    kind="ExternalOutput")
        dbg_ht = nc.dram_tensor("dbg_ht", [128, HC, NLP], f32,
                                kind="ExternalOutput")
        dbg_ax = nc.dram_tensor("dbg_ax", [128, FK, NLP], f32,
                                kind="ExternalOutput")

    def _gather(out_tile, table, idx_sb, b):
        if gather_mode == "multi":
            nc.gpsimd.indirect_dma_start(
                out=out_tile[:], out_offset=None, in_=table[:],
                in_offset=bass.IndirectOffsetOnAxis(
                    ap=idx_sb[:, b * KT:(b + 1) * KT], axis=0))
        else:
            for kt in range(KT):
                nc.gpsimd.indirect_dma_start(
                    out=out_tile[:, kt], out_offset=None, in_=table[:],
                    in_offset=bass.IndirectOffsetOnAxis(
                        ap=idx_sb[:, b * KT + kt:b * KT + kt + 1], axis=0))

    with tile.TileContext(nc) as tc:
        with (
            tc.tile_pool(name="dram", bufs=1, space="DRAM") as dram,
            tc.tile_pool(name="const", bufs=1) as const,
        ):
            # resident constants
            w1_sb = const.tile([128, FK, H], cdt)
            nc.sync.dma_start(out=w1_sb[:], in_=w1_in[:])
            rw1_sb = const.tile([128, FK, H], cdt)
            nc.sync.dma_start(out=rw1_sb[:], in_=rw1_in[:])
            w2_sb = const.tile([128, HC, OUT], cdt)
            nc.sync.dma_start(out=w2_sb[:], in_=w2_in[:])
            rw2b_sb = const.tile([128, HC + 1, OUT], cdt)
            nc.sync.dma_start(out=rw2b_sb[:], in_=rw2b_in[:])
            bias1_sb = const.tile([128, HC], f32)
            nc.sync.dma_start(out=bias1_sb[:], in_=bias1_in[:])
            idx_sb = const.tile([128, B * KT], mybir.dt.int32)
            nc.sync.dma_start(out=idx_sb[:], in_=idx_in[:])
            xt_sb = const.tile([128, FK, NLP], cdt)
            nc.sync.dma_start(out=xt_sb[:], in_=xt_in[:])
            ones_sb = const.tile([128, 128], cdt)
            nc.vector.memset(ones_sb[:], 0.0)
            nc.vector.memset(ones_sb[0:1, :], 1.0)

            for rep in range(reps):
                # internal DRAM (per rep: a Shared collective output must
                # have a single writer instruction)
                hT_d = dram.tile([128, HC, NLP], cdt, tag="hT_d",
                                 name=f"hT_d{rep}")
                hw_loc = dram.tile([NLP, OUT], cdt, tag="hw_loc",
                                   name=f"hw_loc{rep}")
                hw_full = dram.tile([NP, OUT], cdt, addr_space="Shared",
                                    tag="hw_full", name=f"hw_full{rep}")
                # ---------------- phase A: layer 1 + hw ----------------
                with (
                    tc.tile_pool(name=f"xg_pool{rep}", bufs=2) as xg_pool,
                    tc.tile_pool(name=f"sa_pool{rep}", bufs=2) as sa_pool,
                    tc.tile_pool(name=f"axsb_pool{rep}", bufs=2) as axsb_pool,
                    tc.tile_pool(name=f"hstage_pool{rep}", bufs=3) as hstage_pool,
                    tc.tile_pool(name=f"hwsb_pool{rep}", bufs=2) as hwsb_pool,
                    tc.tile_pool(name=f"ax_psum{rep}", bufs=2,
                                 space="PSUM") as ax_psum,
                    tc.tile_pool(name=f"h_psum{rep}", bufs=2,
                                 space="PSUM") as h_psum,
                    tc.tile_pool(name=f"hw_psum{rep}", bufs=2,
                                 space="PSUM") as hw_psum,
                ):
                    for b in range(B):
                        bw = slice(b * 128, (b + 1) * 128)
                        s_sb = sa_pool.tile([128, KT * 128], cdt, tag="s_sb")
                        nc.sync.dma_start(out=s_sb[:], in_=s_in[b])
                        xg = xg_pool.tile([128, KT, F], cdt, tag="xg")
                        _gather(xg, x_table, idx_sb, b)

                        # aggregation in input space: axT[fc] = Xg_chunk.T @ S
                        psum_ax = ax_psum.tile([128, FK, 128], f32,
                                               tag="psum_ax")
                        for fc in range(FK):
                            for kt in range(KT):
                                nc.tensor.matmul(
                                    out=psum_ax[:, fc, :],
                                    lhsT=xg[:, kt, fc * 128:(fc + 1) * 128],
                                    rhs=s_sb[:, kt * 128:(kt + 1) * 128],
                                    start=(kt == 0),
                                    stop=(kt == KT - 1),
                                )
                        axT_sb = axsb_pool.tile([128, FK, 128], cdt,
                                                tag="axT_sb")
                        nc.vector.tensor_copy(out=axT_sb[:], in_=psum_ax[:])
                        if debug_out:
                            nc.gpsimd.dma_start(out=dbg_ax[:, :, bw],
                                                in_=axT_sb[:])

                        # dense: hT = relu(W1.T @ axT + RW1.T @ xT + b1c)
                        h_tiles = []
                        for half in range(2):
                            psum_h = h_psum.tile([128, HC // 2, 128], f32,
                                                 tag="psum_h")
                            for j in range(HC // 2):
                                hc = half * (HC // 2) + j
                                hs = slice(hc * 128, (hc + 1) * 128)
                                for k in range(FK):
                                    nc.tensor.matmul(
                                        out=psum_h[:, j, :],
                                        lhsT=w1_sb[:, k, hs],
                                        rhs=axT_sb[:, k, :],
                                        start=(k == 0), stop=False)
                                for k in range(FK):
                                    nc.tensor.matmul(
                                        out=psum_h[:, j, :],
                                        lhsT=rw1_sb[:, k, hs],
                                        rhs=xt_sb[:, k, bw],
                                        start=False, stop=(k == FK - 1))
                            hT_sb = hstage_pool.tile([128, HC // 2, 128], cdt,
                                                     tag="hT_sb")
                            for j in range(HC // 2):
                                hc = half * (HC // 2) + j
                                nc.scalar.activation(
                                    out=hT_sb[:, j, :], in_=psum_h[:, j, :],
                                    func=mybir.ActivationFunctionType.Relu,
                                    bias=bias1_sb[:, hc:hc + 1], scale=1.0)
                            nc.sync.dma_start(
                                out=hT_d[:, half * (HC // 2):(half + 1) * (HC // 2), bw],
                                in_=hT_sb[:])
                            if debug_out:
                                nc.gpsimd.dma_start(
                                    out=dbg_ht[:, half * (HC // 2):(half + 1) * (HC // 2), bw],
                                    in_=hT_sb[:])
                            h_tiles.append(hT_sb)

                        # hw = h @ W2   (node-major)
                        psum_hw = hw_psum.tile([128, OC, 128], f32,
                                               tag="psum_hw")
                        for oc in range(OC):
                            for half in range(2):
                                for j in range(HC // 2):
                                    hc = half * (HC // 2) + j
                                    nc.tensor.matmul(
                                        out=psum_hw[:, oc, :],
                                        lhsT=h_tiles[half][:, j, :],
                                        rhs=w2_sb[:, hc, oc * 128:(oc + 1) * 128],
                                        start=(half == 0 and j == 0),
                                        stop=(half == 1 and j == HC // 2 - 1))
                        hw_sb = hwsb_pool.tile([128, OUT], cdt, tag="hw_sb")
                        nc.vector.tensor_copy(out=hw_sb[:], in_=psum_hw[:])
                        nc.sync.dma_start(out=hw_loc[bw, :], in_=hw_sb[:])

                # ---------------- all-gather of hw ----------------
                if no_collective:
                    nc.gpsimd.dma_start(out=hw_full[0:NLP, :], in_=hw_loc[:])
                else:
                    nc.gpsimd.collective_compute(
                        "AllGather",
                        mybir.AluOpType.bypass,
                        replica_groups=[list(range(C))],
                        ins=[hw_loc[:].opt()],
                        outs=[hw_full[:].opt()],
                    )
                if debug_out:
                    nc.gpsimd.dma_start(out=dbg_hwfull[:], in_=hw_full[:])

                # ---------------- phase B: layer 2 ----------------
                with (
                    tc.tile_pool(name=f"sb_pool{rep}", bufs=2) as sb_pool,
                    tc.tile_pool(name=f"hwg_pool{rep}", bufs=2) as hwg_pool,
                    tc.tile_pool(name=f"hblk_pool{rep}", bufs=2) as hblk_pool,
                    tc.tile_pool(name=f"osb_pool{rep}", bufs=2) as osb_pool,
                    tc.tile_pool(name=f"o_psum{rep}", bufs=2,
                                 space="PSUM") as o_psum,
                ):
                    for b in range(B):
                        bw = slice(b * 128, (b + 1) * 128)
                        s_sb2 = sb_pool.tile([128, KT * 128], cdt, tag="s_sb2")
                        nc.sync.dma_start(out=s_sb2[:], in_=s_in[b])
                        hwg = hwg_pool.tile([128, KT, OUT], cdt, tag="hwg")
                        _gather(hwg, hw_full, idx_sb, b)
                        hblk = hblk_pool.tile([128, HC, 128], cdt, tag="hblk")
                        nc.sync.dma_start(out=hblk[:], in_=hT_d[:, :, bw])

                        psum_o = o_psum.tile([128, OC, 128], f32, tag="psum_o")
                        for oc in range(OC):
                            ow = slice(oc * 128, (oc + 1) * 128)
                            for kt in range(KT):
                                nc.tensor.matmul(
                                    out=psum_o[:, oc, :],
                                    lhsT=s_sb2[:, kt * 128:(kt + 1) * 128],
                                    rhs=hwg[:, kt, ow],
                                    start=(kt == 0), stop=False)
                            for k in range(HC):
                                nc.tensor.matmul(
                                    out=psum_o[:, oc, :],
                                    lhsT=hblk[:, k, :],
                                    rhs=rw2b_sb[:, k, ow],
                                    start=False, stop=False)
                            nc.tensor.matmul(
                                out=psum_o[:, oc, :],
                                lhsT=ones_sb[:],
                                rhs=rw2b_sb[:, HC, ow],
                                start=False, stop=True)
                        out_sb = osb_pool.tile([128, OUT], f32, tag="out_sb")
                        nc.vector.tensor_copy(out=out_sb[:], in_=psum_o[:])
                        nc.sync.dma_start(out=out_d[bw, :], in_=out_sb[:])

    nc.compile()
    return nc


# ----------------------------------------------------------------------------
# entry points
# ----------------------------------------------------------------------------
def _run(inputs, trace=False, compute=None, trace_kwargs=None):
    compute = compute or COMPUTE
    cdt, np_cdt = _DT[compute]
    x = np.asarray(inputs["x"])
    cfg = _cfg_from_shapes(x, np.asarray(inputs["w1"]),
                           np.asarray(inputs["w2"]))
    in_maps, KT = _preprocess(
        x, inputs["edge_index"], inputs["edge_weight"],
        inputs["w1"], inputs["b1"], inputs["w2"], inputs["b2"],
        inputs["rw1"], inputs["rb1"], inputs["rw2"], inputs["rb2"],
        cfg, np_cdt)

    key = (tuple(sorted(cfg.items())), KT, compute)
    nc = _prog_cache.get(key)
    if nc is None:
        nc = _build(cfg, KT, cdt)
        _prog_cache[key] = nc

    res = run_bass_kernel_spmd(
        nc, in_maps, core_ids=list(range(C)), trace=trace,
        **(trace_kwargs or {}))

    NL, NLP = cfg["NL"], cfg["NLP"]
    out = np.concatenate(
        [res.results[c]["out"][:NL] for c in range(C)], axis=0)
    return np.ascontiguousarray(out.astype(np.float32)), res


def kernel(**inputs) -> np.ndarray:
    out, _ = _run(inputs, trace=False)
    return out


# revision 25
# speedup vs baseline: 168.2910x; 1.1998x over previous
"""Distributed 2-layer GCN (PyG GCNConv-style) on 8 Trainium2 NeuronCores.

Strategy (matches the sharding hint):
  - Nodes are sharded 2500/core (padded to 2560 = 20 blocks of 128).
  - Edges are partitioned by destination node; per (core, dst-block) the
    incoming edges (+ self loops) are packed into uniform KT tiles of 128
    edge slots. The sym-normalization coefficients are precomputed on the
    host into small per-block selection matrices S [128 edge-slots, 128 dst]
    so that scatter-add becomes a TensorE matmul.
  - Layer 1 aggregates in INPUT space (A @ x, width F=512) before the W1
    matmul; layer 2 aggregates AFTER the W2 projection (A @ (h@W2), width
    128). Both choices minimize gathered bytes.
  - The full (padded) x table is replicated to every core's HBM so layer-1
    gathers are local. Layer 2 needs remote h@W2 rows -> one AllGather of
    the [20480, 128] hw table (1.3 MB/core on the wire).
  - Weights are replicated; everything is laid out feature-major on the
    host so the kernel needs zero on-device transposes.

kernel(**inputs) takes the FULL unsharded inputs and returns the FULL
[20000, 128] float32 output.
"""

import math

import numpy as np
import ml_dtypes

import concourse.bass as bass
import concourse.mybir as mybir
import concourse.tile as tile
from concourse import bacc
from concourse.bass_utils import run_bass_kernel_spmd

# ----------------------------------------------------------------------------
# configuration
# ----------------------------------------------------------------------------
C = 8  # cores

# compute dtype for gathers / matmul operands ("bf16" or "f32").
# PSUM accumulation is always fp32. bf16 measured 3.2e-3 rel err vs the
# fp32 reference and roughly halves both HBM traffic and PE time.
COMPUTE = "bf16"

_DT = {
    "bf16": (mybir.dt.bfloat16, ml_dtypes.bfloat16),
    "f32": (mybir.dt.float32, np.float32),
}

_prog_cache: dict = {}


def _cfg_from_shapes(x, w1, w2):
    n, f = x.shape
    h = w1.shape[1]
    out = w2.shape[1]
    assert n % C == 0, n
    nl = n // C                      # real nodes per core
    nlp = ((nl + 127) // 128) * 128  # padded nodes per core
    b = nlp // 128                   # dst blocks per core
    assert f % 128 == 0 and h % 128 == 0 and out % 128 == 0
    return dict(N=n, F=f, H=h, OUT=out, NL=nl, NLP=nlp, B=b, NP=C * nlp,
                FK=f // 128, HC=h // 128, OC=out // 128)


# ----------------------------------------------------------------------------
# host-side preprocessing: graph partition + norm coefficients + layouts
# ----------------------------------------------------------------------------
def _preprocess(x, edge_index, edge_weight, w1, b1, w2, b2, rw1, rb1, rw2, rb2,
                cfg, np_cdt):
    N, F, H, OUT = cfg["N"], cfg["F"], cfg["H"], cfg["OUT"]
    NL, NLP, B, NP = cfg["NL"], cfg["NLP"], cfg["B"], cfg["NP"]
    HC, FK = cfg["HC"], cfg["FK"]

    row = np.asarray(edge_index[0], dtype=np.int64)
    col = np.asarray(edge_index[1], dtype=np.int64)
    ew = np.asarray(edge_weight, dtype=np.float32)

    # symmetric normalization, exactly like the reference (self loop weight 1)
    deg = np.bincount(col, weights=ew.astype(np.float64), minlength=N) + 1.0
    deg = deg.astype(np.float32)
    dis = np.where(deg > 0, 1.0 / np.sqrt(np.where(deg > 0, deg, 1.0)), 0.0)
    dis = dis.astype(np.float32)

    loop = np.arange(N, dtype=np.int64)
    srcs = np.concatenate([row, loop])
    dsts = np.concatenate([col, loop])
    norms = np.concatenate([dis[row] * ew * dis[col], dis * dis])

    # padded node ids: node g lives on core g//NL at local slot g%NL
    src_pad = (srcs // NL) * NLP + (srcs % NL)
    core = dsts // NL
    local = dsts % NL
    blk = local // 128
    dloc = local % 128

    key = (core * B + blk).astype(np.int64)
    order = np.argsort(key, kind="stable")
    key_s = key[order]
    counts = np.bincount(key_s, minlength=C * B)
    starts = np.zeros(C * B, dtype=np.int64)
    np.cumsum(counts[:-1], out=starts[1:])
    pos = np.arange(key_s.size, dtype=np.int64) - starts[key_s]

    KT = max(1, int(math.ceil(counts.max() / 128)))

    src_s = src_pad[order].astype(np.int32)
    core_s = core[order]
    blk_s = blk[order]
    dloc_s = dloc[order]
    norm_s = norms[order]
    kt_s = pos // 128
    p_s = pos % 128

    idx_all = np.zeros((C, 128, B * KT), dtype=np.int32)
    idx_all[core_s, p_s, blk_s * KT + kt_s] = src_s

    # int16 indices for dma_gather: slot i of block b -> [i%16, b*KT*8 + i//16],
    # replicated across the 8 groups of 16 partitions
    assert NP < 2 ** 15
    idx16 = np.zeros((C, 16, B * KT * 8), dtype=np.int16)
    slot = kt_s * 128 + p_s
    idx16[core_s, slot % 16, blk_s * (KT * 8) + slot // 16] = \
        src_s.astype(np.int16)
    idx16_all = np.tile(idx16, (1, 8, 1))

    S_all = np.zeros((C, B, 128, KT * 128), dtype=np.float32)
    S_all[core_s, blk_s, p_s, kt_s * 128 + dloc_s] = norm_s
    S_all = S_all.astype(np_cdt)

    # padded, replicated x table [NP, F]
    x = np.asarray(x, dtype=np.float32)
    x_table = np.zeros((NP, F), dtype=np.float32)
    x_table.reshape(C, NLP, F)[:, :NL] = x.reshape(C, NL, F)
    x_table = x_table.astype(np_cdt)

    # feature-major x per core: xT[p, k, n] = x_core[n, k*128+p]
    xT_all = np.ascontiguousarray(
        x_table.reshape(C, NLP, FK, 128).transpose(0, 3, 2, 1))

    w1 = np.asarray(w1, np.float32)
    rw1 = np.asarray(rw1, np.float32)
    w2 = np.asarray(w2, np.float32)
    rw2 = np.asarray(rw2, np.float32)
    b1c = (np.asarray(b1, np.float32) + np.asarray(rb1, np.float32))
    b2c = (np.asarray(b2, np.float32) + np.asarray(rb2, np.float32))

    # [128, FK, H] : w1_in[p, k, j] = w1[k*128+p, j]
    w1_in = np.ascontiguousarray(
        w1.reshape(FK, 128, H).transpose(1, 0, 2)).astype(np_cdt)
    rw1_in = np.ascontiguousarray(
        rw1.reshape(FK, 128, H).transpose(1, 0, 2)).astype(np_cdt)
    w2_in = np.ascontiguousarray(
        w2.reshape(HC, 128, OUT).transpose(1, 0, 2)).astype(np_cdt)
    rw2b_in = np.zeros((128, HC + 1, OUT), dtype=np.float32)
    rw2b_in[:, :HC] = rw2.reshape(HC, 128, OUT).transpose(1, 0, 2)
    rw2b_in[0, HC, :] = b2c
    rw2b_in = rw2b_in.astype(np_cdt)

    bias1_in = np.ascontiguousarray(b1c.reshape(HC, 128).T).astype(np.float32)

    in_maps = []
    for c in range(C):
        in_maps.append({
            "x_table": x_table,
            "idx_in": np.ascontiguousarray(idx_all[c]),
            "idx16_in": np.ascontiguousarray(idx16_all[c]),
            "s_in": np.ascontiguousarray(S_all[c]),
            "xt_in": np.ascontiguousarray(xT_all[c]),
            "w1_in": w1_in,
            "rw1_in": rw1_in,
            "w2_in": w2_in,
            "rw2b_in": rw2b_in,
            "bias1_in": bias1_in,
        })
    return in_maps, KT


# ----------------------------------------------------------------------------
# device program
# ----------------------------------------------------------------------------
def _build(cfg, KT, cdt, gather_mode="dg", debug_out=False, reps=1,
           no_collective=False):
    F, H, OUT = cfg["F"], cfg["H"], cfg["OUT"]
    NLP, B, NP = cfg["NLP"], cfg["B"], cfg["NP"]
    FK, HC, OC = cfg["FK"], cfg["HC"], cfg["OC"]
    f32 = mybir.dt.float32

    # 64 KB SWDGE scratch = 4096-descriptor ring: a whole-block gather
    # (KT*128 descriptors in one instruction) must fit, else HW corrupts.
    nc = bacc.Bacc("TRN2", target_bir_lowering=False, debug=False,
                   enable_asserts=False, num_devices=C,
                   dynamic_dma_scratch_size=65536)

    x_table = nc.dram_tensor("x_table", [NP, F], cdt, kind="ExternalInput")
    idx_in = nc.dram_tensor("idx_in", [128, B * KT], mybir.dt.int32,
                            kind="ExternalInput")
    idx16_in = nc.dram_tensor("idx16_in", [128, B * KT * 8], mybir.dt.int16,
                              kind="ExternalInput")
    s_in = nc.dram_tensor("s_in", [B, 128, KT * 128], cdt,
                          kind="ExternalInput")
    xt_in = nc.dram_tensor("xt_in", [128, FK, NLP], cdt, kind="ExternalInput")
    w1_in = nc.dram_tensor("w1_in", [128, FK, H], cdt, kind="ExternalInput")
    rw1_in = nc.dram_tensor("rw1_in", [128, FK, H], cdt, kind="ExternalInput")
    w2_in = nc.dram_tensor("w2_in", [128, HC, OUT], cdt, kind="ExternalInput")
    rw2b_in = nc.dram_tensor("rw2b_in", [128, HC + 1, OUT], cdt,
                             kind="ExternalInput")
    bias1_in = nc.dram_tensor("bias1_in", [128, HC], f32,
                              kind="ExternalInput")
    out_d = nc.dram_tensor("out", [NLP, OUT], f32, kind="ExternalOutput")
    if debug_out:
        dbg_hwfull = nc.dram_tensor("dbg_hwfull", [NP, OUT], f32,
                                    kind="ExternalOutput")
        dbg_ht = nc.dram_tensor("dbg_ht", [128, HC, NLP], f32,
                                kind="ExternalOutput")
        dbg_ax = nc.dram_tensor("dbg_ax", [128, FK, NLP], f32,
                                kind="ExternalOutput")

    # "dg": one bulk InstDMAGatherAnt per block (KT*128 rows, needs
    # single_packet=False). "per_kt": one indirect DMA per 128 rows —
    # multi-column indirect offset APs are broken on HW, [128,1] only.
    def _gather(out_tile, table, idx_sb, idx16_sb, b, elem):
        if gather_mode == "dg":
            nc.gpsimd.dma_gather(
                out_ap=out_tile[:], in_ap=table[:],
                idxs_ap=idx16_sb[:, b * KT * 8:(b + 1) * KT * 8],
                num_idxs=KT * 128, num_idxs_reg=KT * 128,
                elem_size=elem, single_packet=False)
        else:
            for kt in range(KT):
                nc.gpsimd.indirect_dma_start(
                    out=out_tile[:, kt], out_offset=None, in_=table[:],
                    in_offset=bass.IndirectOffsetOnAxis(
                        ap=idx_sb[:, b * KT + kt:b * KT + kt + 1], axis=0))

    with tile.TileContext(nc) as tc:
        with (
            tc.tile_pool(name="dram", bufs=1, space="DRAM") as dram,
            tc.tile_pool(name="const", bufs=1) as const,
        ):
            # resident constants
            w1_sb = const.tile([128, FK, H], cdt)
            nc.sync.dma_start(out=w1_sb[:], in_=w1_in[:])
            rw1_sb = const.tile([128, FK, H], cdt)
            nc.sync.dma_start(out=rw1_sb[:], in_=rw1_in[:])
            w2_sb = const.tile([128, HC, OUT], cdt)
            nc.sync.dma_start(out=w2_sb[:], in_=w2_in[:])
            rw2b_sb = const.tile([128, HC + 1, OUT], cdt)
            nc.sync.dma_start(out=rw2b_sb[:], in_=rw2b_in[:])
            bias1_sb = const.tile([128, HC], f32)
            nc.sync.dma_start(out=bias1_sb[:], in_=bias1_in[:])
            idx_sb = const.tile([128, B * KT], mybir.dt.int32)
            nc.sync.dma_start(out=idx_sb[:], in_=idx_in[:])
            idx16_sb = const.tile([128, B * KT * 8], mybir.dt.int16)
            nc.sync.dma_start(out=idx16_sb[:], in_=idx16_in[:])
            xt_sb = const.tile([128, FK, NLP], cdt)
            nc.sync.dma_start(out=xt_sb[:], in_=xt_in[:])
            ones_sb = const.tile([128, 128], cdt)
            nc.vector.memset(ones_sb[:], 0.0)
            nc.vector.memset(ones_sb[0:1, :], 1.0)

            for rep in range(reps):
                # internal DRAM (per rep: a Shared collective output must
                # have a single writer instruction)
                hT_d = dram.tile([128, HC, NLP], cdt, tag="hT_d",
                                 name=f"hT_d{rep}")
                hw_loc = dram.tile([NLP, OUT], cdt, tag="hw_loc",
                                   name=f"hw_loc{rep}")
                hw_full = dram.tile([NP, OUT], cdt, addr_space="Shared",
                                    tag="hw_full", name=f"hw_full{rep}")
                # ---------------- phase A: layer 1 + hw ----------------
                with (
                    tc.tile_pool(name=f"xg_pool{rep}", bufs=2) as xg_pool,
                    tc.tile_pool(name=f"sa_pool{rep}", bufs=2) as sa_pool,
                    tc.tile_pool(name=f"axsb_pool{rep}", bufs=2) as axsb_pool,
                    tc.tile_pool(name=f"hstage_pool{rep}", bufs=3) as hstage_pool,
                    tc.tile_pool(name=f"hwsb_pool{rep}", bufs=2) as hwsb_pool,
                    tc.tile_pool(name=f"ax_psum{rep}", bufs=2,
                                 space="PSUM") as ax_psum,
                    tc.tile_pool(name=f"h_psum{rep}", bufs=2,
                                 space="PSUM") as h_psum,
                    tc.tile_pool(name=f"hw_psum{rep}", bufs=2,
                                 space="PSUM") as hw_psum,
                ):
                    for b in range(B):
                        bw = slice(b * 128, (b + 1) * 128)
                        s_sb = sa_pool.tile([128, KT * 128], cdt, tag="s_sb")
                        nc.sync.dma_start(out=s_sb[:], in_=s_in[b])
                        xg = xg_pool.tile([128, KT, F], cdt, tag="xg")
                        _gather(xg, x_table, idx_sb, idx16_sb, b, F)

                        # aggregation in input space: axT[fc] = Xg_chunk.T @ S
                        psum_ax = ax_psum.tile([128, FK, 128], f32,
                                               tag="psum_ax")
                        for fc in range(FK):
                            for kt in range(KT):
                                nc.tensor.matmul(
                                    out=psum_ax[:, fc, :],
                                    lhsT=xg[:, kt, fc * 128:(fc + 1) * 128],
                                    rhs=s_sb[:, kt * 128:(kt + 1) * 128],
                                    start=(kt == 0),
                                    stop=(kt == KT - 1),
                                )
                        axT_sb = axsb_pool.tile([128, FK, 128], cdt,
                                                tag="axT_sb")
                        nc.vector.tensor_copy(out=axT_sb[:], in_=psum_ax[:])
                        if debug_out:
                            nc.gpsimd.dma_start(out=dbg_ax[:, :, bw],
                                                in_=axT_sb[:])

                        # dense: hT = relu(W1.T @ axT + RW1.T @ xT + b1c)
                        h_tiles = []
                        for half in range(2):
                            psum_h = h_psum.tile([128, HC // 2, 128], f32,
                                                 tag="psum_h")
                            for j in range(HC // 2):
                                hc = half * (HC // 2) + j
                                hs = slice(hc * 128, (hc + 1) * 128)
                                for k in range(FK):
                                    nc.tensor.matmul(
                                        out=psum_h[:, j, :],
                                        lhsT=w1_sb[:, k, hs],
                                        rhs=axT_sb[:, k, :],
                                        start=(k == 0), stop=False)
                                for k in range(FK):
                                    nc.tensor.matmul(
                                        out=psum_h[:, j, :],
                                        lhsT=rw1_sb[:, k, hs],
                                        rhs=xt_sb[:, k, bw],
                                        start=False, stop=(k == FK - 1))
                            hT_sb = hstage_pool.tile([128, HC // 2, 128], cdt,
                                                     tag="hT_sb")
                            for j in range(HC // 2):
                                hc = half * (HC // 2) + j
                                nc.scalar.activation(
                                    out=hT_sb[:, j, :], in_=psum_h[:, j, :],
                                    func=mybir.ActivationFunctionType.Relu,
                                    bias=bias1_sb[:, hc:hc + 1], scale=1.0)
                            nc.sync.dma_start(
                                out=hT_d[:, half * (HC // 2):(half + 1) * (HC // 2), bw],
                                in_=hT_sb[:])
                            if debug_out:
                                nc.gpsimd.dma_start(
                                    out=dbg_ht[:, half * (HC // 2):(half + 1) * (HC // 2), bw],
                                    in_=hT_sb[:])
                            h_tiles.append(hT_sb)

                        # hw = h @ W2   (node-major)
                        psum_hw = hw_psum.tile([128, OC, 128], f32,
                                               tag="psum_hw")
                        for oc in range(OC):
                            for half in range(2):
                                for j in range(HC // 2):
                                    hc = half * (HC // 2) + j
                                    nc.tensor.matmul(
                                        out=psum_hw[:, oc, :],
                                        lhsT=h_tiles[half][:, j, :],
                                        rhs=w2_sb[:, hc, oc * 128:(oc + 1) * 128],
                                        start=(half == 0 and j == 0),
                                        stop=(half == 1 and j == HC // 2 - 1))
                        hw_sb = hwsb_pool.tile([128, OUT], cdt, tag="hw_sb")
                        nc.vector.tensor_copy(out=hw_sb[:], in_=psum_hw[:])
                        nc.sync.dma_start(out=hw_loc[bw, :], in_=hw_sb[:])

                # ---------------- all-gather of hw ----------------
                if no_collective:
                    nc.gpsimd.dma_start(out=hw_full[0:NLP, :], in_=hw_loc[:])
                else:
                    nc.gpsimd.collective_compute(
                        "AllGather",
                        mybir.AluOpType.bypass,
                        replica_groups=[list(range(C))],
                        ins=[hw_loc[:].opt()],
                        outs=[hw_full[:].opt()],
                    )
                if debug_out:
                    nc.gpsimd.dma_start(out=dbg_hwfull[:], in_=hw_full[:])

                # ---------------- phase B: layer 2 ----------------
                with (
                    tc.tile_pool(name=f"sb_pool{rep}", bufs=2) as sb_pool,
                    tc.tile_pool(name=f"hwg_pool{rep}", bufs=2) as hwg_pool,
                    tc.tile_pool(name=f"hblk_pool{rep}", bufs=2) as hblk_pool,
                    tc.tile_pool(name=f"osb_pool{rep}", bufs=2) as osb_pool,
                    tc.tile_pool(name=f"o_psum{rep}", bufs=2,
                                 space="PSUM") as o_psum,
                ):
                    for b in range(B):
                        bw = slice(b * 128, (b + 1) * 128)
                        s_sb2 = sb_pool.tile([128, KT * 128], cdt, tag="s_sb2")
                        nc.sync.dma_start(out=s_sb2[:], in_=s_in[b])
                        hwg = hwg_pool.tile([128, KT, OUT], cdt, tag="hwg")
                        _gather(hwg, hw_full, idx_sb, idx16_sb, b, OUT)
                        hblk = hblk_pool.tile([128, HC, 128], cdt, tag="hblk")
                        nc.sync.dma_start(out=hblk[:], in_=hT_d[:, :, bw])

                        psum_o = o_psum.tile([128, OC, 128], f32, tag="psum_o")
                        for oc in range(OC):
                            ow = slice(oc * 128, (oc + 1) * 128)
                            for kt in range(KT):
                                nc.tensor.matmul(
                                    out=psum_o[:, oc, :],
                                    lhsT=s_sb2[:, kt * 128:(kt + 1) * 128],
                                    rhs=hwg[:, kt, ow],
                                    start=(kt == 0), stop=False)
                            for k in range(HC):
                                nc.tensor.matmul(
                                    out=psum_o[:, oc, :],
                                    lhsT=hblk[:, k, :],
                                    rhs=rw2b_sb[:, k, ow],
                                    start=False, stop=False)
                            nc.tensor.matmul(
                                out=psum_o[:, oc, :],
                                lhsT=ones_sb[:],
                                rhs=rw2b_sb[:, HC, ow],
                                start=False, stop=True)
                        out_sb = osb_pool.tile([128, OUT], f32, tag="out_sb")
                        nc.vector.tensor_copy(out=out_sb[:], in_=psum_o[:])
                        nc.sync.dma_start(out=out_d[bw, :], in_=out_sb[:])

    nc.compile()
    return nc


# ----------------------------------------------------------------------------
# entry points
# ----------------------------------------------------------------------------
def _run(inputs, trace=False, compute=None, trace_kwargs=None):
    compute = compute or COMPUTE
    cdt, np_cdt = _DT[compute]
    x = np.asarray(inputs["x"])
    cfg = _cfg_from_shapes(x, np.asarray(inputs["w1"]),
                           np.asarray(inputs["w2"]))
    in_maps, KT = _preprocess(
        x, inputs["edge_index"], inputs["edge_weight"],
        inputs["w1"], inputs["b1"], inputs["w2"], inputs["b2"],
        inputs["rw1"], inputs["rb1"], inputs["rw2"], inputs["rb2"],
        cfg, np_cdt)

    key = (tuple(sorted(cfg.items())), KT, compute)
    nc = _prog_cache.get(key)
    if nc is None:
        nc = _build(cfg, KT, cdt)
        _prog_cache[key] = nc

    res = run_bass_kernel_spmd(
        nc, in_maps, core_ids=list(range(C)), trace=trace,
        **(trace_kwargs or {}))

    NL, NLP = cfg["NL"], cfg["NLP"]
    out = np.concatenate(
        [res.results[c]["out"][:NL] for c in range(C)], axis=0)
    return np.ascontiguousarray(out.astype(np.float32)), res


def kernel(**inputs) -> np.ndarray:
    out, _ = _run(inputs, trace=False)
    return out


# revision 48
# speedup vs baseline: 229.4009x; 1.3631x over previous
"""Distributed 2-layer GCN (PyG GCNConv-style) on 8 Trainium2 NeuronCores.

Strategy (matches the sharding hint):
  - Nodes are sharded 2500/core (padded to 2560 = 20 blocks of 128).
  - Edges are partitioned by destination node; per (core, dst-block) the
    incoming edges (+ self loops) are packed into uniform KT tiles of 128
    edge slots. The sym-normalization coefficients are precomputed on the
    host into small per-block selection matrices S [128 edge-slots, 128 dst]
    so that scatter-add becomes a TensorE matmul.
  - Layer 1 aggregates in INPUT space (A @ x, width F=512) before the W1
    matmul; layer 2 aggregates AFTER the W2 projection (A @ (h@W2), width
    128). Both choices minimize gathered bytes.
  - The full (padded) x table is replicated to every core's HBM so layer-1
    gathers are local. Layer 2 needs remote h@W2 rows -> one AllGather of
    the [20480, 128] hw table (1.3 MB/core on the wire).
  - Weights are replicated; everything is laid out feature-major on the
    host so the kernel needs zero on-device transposes.

kernel(**inputs) takes the FULL unsharded inputs and returns the FULL
[20000, 128] float32 output.
"""

import math

import numpy as np
import ml_dtypes

import concourse.bass as bass
import concourse.mybir as mybir
import concourse.tile as tile
from concourse import bacc
from concourse.bass_utils import run_bass_kernel_spmd

# ----------------------------------------------------------------------------
# configuration
# ----------------------------------------------------------------------------
C = 8  # cores

# compute dtype for gathers / matmul operands ("bf16" or "f32").
# PSUM accumulation is always fp32. bf16 measured 3.2e-3 rel err vs the
# fp32 reference and roughly halves both HBM traffic and PE time.
COMPUTE = "bf16"

_DT = {
    "bf16": (mybir.dt.bfloat16, ml_dtypes.bfloat16),
    "f32": (mybir.dt.float32, np.float32),
}

_prog_cache: dict = {}


def _cfg_from_shapes(x, w1, w2):
    n, f = x.shape
    h = w1.shape[1]
    out = w2.shape[1]
    assert n % C == 0, n
    nl = n // C                      # real nodes per core
    nlp = ((nl + 127) // 128) * 128  # padded nodes per core
    b = nlp // 128                   # dst blocks per core
    assert f % 128 == 0 and h % 128 == 0 and out % 128 == 0
    return dict(N=n, F=f, H=h, OUT=out, NL=nl, NLP=nlp, B=b, NP=C * nlp,
                FK=f // 128, HC=h // 128, OC=out // 128)


# ----------------------------------------------------------------------------
# host-side preprocessing: graph partition + norm coefficients + layouts
# ----------------------------------------------------------------------------
def _preprocess(x, edge_index, edge_weight, w1, b1, w2, b2, rw1, rb1, rw2, rb2,
                cfg, np_cdt):
    N, F, H, OUT = cfg["N"], cfg["F"], cfg["H"], cfg["OUT"]
    NL, NLP, B, NP = cfg["NL"], cfg["NLP"], cfg["B"], cfg["NP"]
    HC, FK = cfg["HC"], cfg["FK"]

    row = np.asarray(edge_index[0], dtype=np.int64)
    col = np.asarray(edge_index[1], dtype=np.int64)
    ew = np.asarray(edge_weight, dtype=np.float32)

    # symmetric normalization, exactly like the reference (self loop weight 1)
    deg = np.bincount(col, weights=ew.astype(np.float64), minlength=N) + 1.0
    deg = deg.astype(np.float32)
    dis = np.where(deg > 0, 1.0 / np.sqrt(np.where(deg > 0, deg, 1.0)), 0.0)
    dis = dis.astype(np.float32)

    loop = np.arange(N, dtype=np.int64)
    srcs = np.concatenate([row, loop])
    dsts = np.concatenate([col, loop])
    norms = np.concatenate([dis[row] * ew * dis[col], dis * dis])

    # padded node ids: node g lives on core g//NL at local slot g%NL
    src_pad = (srcs // NL) * NLP + (srcs % NL)
    core = dsts // NL
    local = dsts % NL
    blk = local // 128
    dloc = local % 128

    key = (core * B + blk).astype(np.int64)
    order = np.argsort(key, kind="stable")
    key_s = key[order]
    counts = np.bincount(key_s, minlength=C * B)
    starts = np.zeros(C * B, dtype=np.int64)
    np.cumsum(counts[:-1], out=starts[1:])
    pos = np.arange(key_s.size, dtype=np.int64) - starts[key_s]

    KT = max(1, int(math.ceil(counts.max() / 128)))

    src_s = src_pad[order].astype(np.int32)
    core_s = core[order]
    blk_s = blk[order]
    dloc_s = dloc[order]
    norm_s = norms[order]
    kt_s = pos // 128
    p_s = pos % 128

    idx_all = np.zeros((C, 128, B * KT), dtype=np.int32)
    idx_all[core_s, p_s, blk_s * KT + kt_s] = src_s

    # int16 indices for dma_gather: slot i of block b -> [i%16, b*KT*8 + i//16],
    # replicated across the 8 groups of 16 partitions
    assert NP < 2 ** 15
    idx16 = np.zeros((C, 16, B * KT * 8), dtype=np.int16)
    slot = kt_s * 128 + p_s
    idx16[core_s, slot % 16, blk_s * (KT * 8) + slot // 16] = \
        src_s.astype(np.int16)
    idx16_all = np.tile(idx16, (1, 8, 1))

    # partition-major: S_all[c, p, b, kt*128+d] so one contiguous DMA loads
    # a core's whole S into SBUF
    S_all = np.zeros((C, 128, B, KT * 128), dtype=np.float32)
    S_all[core_s, p_s, blk_s, kt_s * 128 + dloc_s] = norm_s
    S_all = S_all.astype(np_cdt)

    # padded, replicated x table [NP, F]
    x = np.asarray(x, dtype=np.float32)
    x_table = np.zeros((NP, F), dtype=np.float32)
    x_table.reshape(C, NLP, F)[:, :NL] = x.reshape(C, NL, F)
    x_table = x_table.astype(np_cdt)

    # feature-major x per core: xT[p, k, n] = x_core[n, k*128+p]
    xT_all = np.ascontiguousarray(
        x_table.reshape(C, NLP, FK, 128).transpose(0, 3, 2, 1))

    w1 = np.asarray(w1, np.float32)
    rw1 = np.asarray(rw1, np.float32)
    w2 = np.asarray(w2, np.float32)
    rw2 = np.asarray(rw2, np.float32)
    b1c = (np.asarray(b1, np.float32) + np.asarray(rb1, np.float32))
    b2c = (np.asarray(b2, np.float32) + np.asarray(rb2, np.float32))

    # [128, FK, H] : w1_in[p, k, j] = w1[k*128+p, j]
    w1_in = np.ascontiguousarray(
        w1.reshape(FK, 128, H).transpose(1, 0, 2)).astype(np_cdt)
    rw1_in = np.ascontiguousarray(
        rw1.reshape(FK, 128, H).transpose(1, 0, 2)).astype(np_cdt)
    w2_in = np.ascontiguousarray(
        w2.reshape(HC, 128, OUT).transpose(1, 0, 2)).astype(np_cdt)
    rw2b_in = np.zeros((128, HC + 1, OUT), dtype=np.float32)
    rw2b_in[:, :HC] = rw2.reshape(HC, 128, OUT).transpose(1, 0, 2)
    rw2b_in[0, HC, :] = b2c
    rw2b_in = rw2b_in.astype(np_cdt)

    bias1_in = np.ascontiguousarray(b1c.reshape(HC, 128).T).astype(np.float32)

    in_maps = []
    for c in range(C):
        in_maps.append({
            "x_table": x_table,
            "idx_in": np.ascontiguousarray(idx_all[c]),
            "idx16_in": np.ascontiguousarray(idx16_all[c]),
            "s_in": np.ascontiguousarray(S_all[c]),
            "xt_in": np.ascontiguousarray(xT_all[c]),
            "w1_in": w1_in,
            "rw1_in": rw1_in,
            "w2_in": w2_in,
            "rw2b_in": rw2b_in,
            "bias1_in": bias1_in,
        })
    return in_maps, KT


# ----------------------------------------------------------------------------
# device program
# ----------------------------------------------------------------------------
def _build(cfg, KT, cdt, gather_mode="dg", debug_out=False, reps=1,
           no_collective=False):
    F, H, OUT = cfg["F"], cfg["H"], cfg["OUT"]
    NLP, B, NP = cfg["NLP"], cfg["B"], cfg["NP"]
    FK, HC, OC = cfg["FK"], cfg["HC"], cfg["OC"]
    f32 = mybir.dt.float32

    # 64 KB SWDGE scratch = 4096-descriptor ring: a whole-block gather
    # (KT*128 descriptors in one instruction) must fit, else HW corrupts.
    nc = bacc.Bacc("TRN2", target_bir_lowering=False, debug=False,
                   enable_asserts=False, num_devices=C,
                   dynamic_dma_scratch_size=65536, num_swdge_queues=2)

    x_table = nc.dram_tensor("x_table", [NP, F], cdt, kind="ExternalInput")
    idx_in = nc.dram_tensor("idx_in", [128, B * KT], mybir.dt.int32,
                            kind="ExternalInput")
    idx16_in = nc.dram_tensor("idx16_in", [128, B * KT * 8], mybir.dt.int16,
                              kind="ExternalInput")
    s_in = nc.dram_tensor("s_in", [128, B, KT * 128], cdt,
                          kind="ExternalInput")
    xt_in = nc.dram_tensor("xt_in", [128, FK, NLP], cdt, kind="ExternalInput")
    w1_in = nc.dram_tensor("w1_in", [128, FK, H], cdt, kind="ExternalInput")
    rw1_in = nc.dram_tensor("rw1_in", [128, FK, H], cdt, kind="ExternalInput")
    w2_in = nc.dram_tensor("w2_in", [128, HC, OUT], cdt, kind="ExternalInput")
    rw2b_in = nc.dram_tensor("rw2b_in", [128, HC + 1, OUT], cdt,
                             kind="ExternalInput")
    bias1_in = nc.dram_tensor("bias1_in", [128, HC], f32,
                              kind="ExternalInput")
    out_d = nc.dram_tensor("out", [NLP, OUT], f32, kind="ExternalOutput")
    if debug_out:
        dbg_hwfull = nc.dram_tensor("dbg_hwfull", [NP, OUT], f32,
                                    kind="ExternalOutput")
        dbg_ht = nc.dram_tensor("dbg_ht", [128, HC, NLP], f32,
                                kind="ExternalOutput")
        dbg_ax = nc.dram_tensor("dbg_ax", [128, FK, NLP], f32,
                                kind="ExternalOutput")

    # "dg": one bulk InstDMAGatherAnt per block (KT*128 rows, needs
    # single_packet=False). "per_kt": one indirect DMA per 128 rows —
    # multi-column indirect offset APs are broken on HW, [128,1] only.
    def _gather(out_tile, table, idx_sb, idx16_sb, b, elem):
        if gather_mode == "dg":
            # two ops per block: halves the gather latency the first
            # aggregation matmuls wait on, and pipelines the Q7 ucode
            h0 = KT // 2
            for lo, hi in ((0, h0), (h0, KT)):
                nc.gpsimd.dma_gather(
                    out_ap=out_tile[:, lo:hi], in_ap=table[:],
                    idxs_ap=idx16_sb[:, b * KT * 8 + lo * 8:
                                     b * KT * 8 + hi * 8],
                    num_idxs=(hi - lo) * 128, num_idxs_reg=(hi - lo) * 128,
                    elem_size=elem, single_packet=False,
                    queue_num=b % 2)
        else:
            for kt in range(KT):
                nc.gpsimd.indirect_dma_start(
                    out=out_tile[:, kt], out_offset=None, in_=table[:],
                    in_offset=bass.IndirectOffsetOnAxis(
                        ap=idx_sb[:, b * KT + kt:b * KT + kt + 1], axis=0))

    with tile.TileContext(nc) as tc:
        with (
            tc.tile_pool(name="dram", bufs=1, space="DRAM") as dram,
            tc.tile_pool(name="const", bufs=1) as const,
        ):
            # resident constants
            w1_sb = const.tile([128, FK, H], cdt)
            nc.sync.dma_start(out=w1_sb[:], in_=w1_in[:])
            rw1_sb = const.tile([128, FK, H], cdt)
            nc.sync.dma_start(out=rw1_sb[:], in_=rw1_in[:])
            w2_sb = const.tile([128, HC, OUT], cdt)
            nc.sync.dma_start(out=w2_sb[:], in_=w2_in[:])
            rw2b_sb = const.tile([128, HC + 1, OUT], cdt)
            nc.sync.dma_start(out=rw2b_sb[:], in_=rw2b_in[:])
            bias1_sb = const.tile([128, HC], f32)
            nc.sync.dma_start(out=bias1_sb[:], in_=bias1_in[:])
            if gather_mode == "dg":
                idx_sb = None
            else:
                idx_sb = const.tile([128, B * KT], mybir.dt.int32)
                nc.sync.dma_start(out=idx_sb[:], in_=idx_in[:])
            idx16_sb = const.tile([128, B * KT * 8], mybir.dt.int16)
            nc.sync.dma_start(out=idx16_sb[:], in_=idx16_in[:])
            ones_sb = const.tile([128, 128], cdt)
            nc.vector.memset(ones_sb[:], 0.0)
            nc.vector.memset(ones_sb[0:1, :], 1.0)

            # bf16 fits S (7.9 MB) and hT (5.2 MB) resident in SBUF, killing
            # the phase-B S reload and the hT DRAM round-trip; f32 streams.
            resident = cdt == mybir.dt.bfloat16
            if resident:
                s_all_sb = const.tile([128, B, KT * 128], cdt)
                nc.sync.dma_start(out=s_all_sb[:], in_=s_in[:])
                hT_all = const.tile([128, HC, NLP], cdt)

            for rep in range(reps):
                # internal DRAM (per rep: a Shared collective output must
                # have a single writer instruction)
                if not resident:
                    hT_d = dram.tile([128, HC, NLP], cdt, tag="hT_d",
                                     name=f"hT_d{rep}")
                hw_loc = dram.tile([NLP, OUT], cdt, tag="hw_loc",
                                   name=f"hw_loc{rep}")
                hw_full = dram.tile([NP, OUT], cdt, addr_space="Shared",
                                    tag="hw_full", name=f"hw_full{rep}")
                # ---------------- phase A: layer 1 + hw ----------------
                with (
                    tc.tile_pool(name=f"xg_pool{rep}", bufs=2) as xg_pool,
                    tc.tile_pool(name=f"sa_pool{rep}", bufs=3) as sa_pool,
                    tc.tile_pool(name=f"axsb_pool{rep}", bufs=2) as axsb_pool,
                    tc.tile_pool(name=f"hstage_pool{rep}", bufs=3) as hstage_pool,
                    tc.tile_pool(name=f"hwsb_pool{rep}", bufs=2) as hwsb_pool,
                    tc.tile_pool(name=f"ax_psum{rep}", bufs=2,
                                 space="PSUM") as ax_psum,
                    tc.tile_pool(name=f"h_psum{rep}", bufs=2,
                                 space="PSUM") as h_psum,
                    tc.tile_pool(name=f"hw_psum{rep}", bufs=2,
                                 space="PSUM") as hw_psum,
                ):
                    for b in range(B):
                        bw = slice(b * 128, (b + 1) * 128)
                        if resident:
                            s_rhs = lambda lo, hi, b=b: s_all_sb[:, b, lo:hi]
                        else:
                            s_tile = sa_pool.tile([128, KT * 128], cdt,
                                                  tag="s_sb")
                            nc.sync.dma_start(out=s_tile[:],
                                              in_=s_in[:, b, :])
                            s_rhs = lambda lo, hi, t=s_tile: t[:, lo:hi]
                        xg = xg_pool.tile([128, KT, F], cdt, tag="xg")
                        _gather(xg, x_table, idx_sb, idx16_sb, b, F)
                        xt_tile = sa_pool.tile([128, FK, 128], cdt,
                                               tag="xt_tile")
                        nc.sync.dma_start(out=xt_tile[:],
                                          in_=xt_in[:, :, bw])

                        # aggregation in input space: axT[fc] = Xg_chunk.T @ S
                        psum_ax = ax_psum.tile([128, FK, 128], f32,
                                               tag="psum_ax")
                        for fc in range(FK):
                            for kt in range(KT):
                                nc.tensor.matmul(
                                    out=psum_ax[:, fc, :],
                                    lhsT=xg[:, kt, fc * 128:(fc + 1) * 128],
                                    rhs=s_rhs(kt * 128, (kt + 1) * 128),
                                    start=(kt == 0),
                                    stop=(kt == KT - 1),
                                )
                        axT_sb = axsb_pool.tile([128, FK, 128], cdt,
                                                tag="axT_sb")
                        nc.vector.tensor_copy(out=axT_sb[:], in_=psum_ax[:])
                        if debug_out:
                            nc.gpsimd.dma_start(out=dbg_ax[:, :, bw],
                                                in_=axT_sb[:])

                        # dense: hT = relu(W1.T @ axT + RW1.T @ xT + b1c)
                        h_tiles = []
                        for half in range(2):
                            psum_h = h_psum.tile([128, HC // 2, 128], f32,
                                                 tag="psum_h")
                            for j in range(HC // 2):
                                hc = half * (HC // 2) + j
                                hs = slice(hc * 128, (hc + 1) * 128)
                                for k in range(FK):
                                    nc.tensor.matmul(
                                        out=psum_h[:, j, :],
                                        lhsT=w1_sb[:, k, hs],
                                        rhs=axT_sb[:, k, :],
                                        start=(k == 0), stop=False)
                                for k in range(FK):
                                    nc.tensor.matmul(
                                        out=psum_h[:, j, :],
                                        lhsT=rw1_sb[:, k, hs],
                                        rhs=xt_tile[:, k, :],
                                        start=False, stop=(k == FK - 1))
                            hw0 = half * (HC // 2)
                            if resident:
                                act_out = (lambda j, hw0=hw0, bw=bw:
                                           hT_all[:, hw0 + j, bw])
                                h_tiles.append(lambda j, hw0=hw0, bw=bw:
                                               hT_all[:, hw0 + j, bw])
                            else:
                                hT_stage = hstage_pool.tile(
                                    [128, HC // 2, 128], cdt, tag="hT_sb")
                                act_out = (lambda j, t=hT_stage: t[:, j, :])
                                h_tiles.append(
                                    lambda j, t=hT_stage: t[:, j, :])
                            for j in range(HC // 2):
                                hc = hw0 + j
                                nc.scalar.activation(
                                    out=act_out(j), in_=psum_h[:, j, :],
                                    func=mybir.ActivationFunctionType.Relu,
                                    bias=bias1_sb[:, hc:hc + 1], scale=1.0)
                            if not resident:
                                nc.sync.dma_start(
                                    out=hT_d[:, hw0:hw0 + HC // 2, bw],
                                    in_=hT_stage[:])
                            if debug_out and resident:
                                pass

                        # hw = h @ W2   (node-major)
                        psum_hw = hw_psum.tile([128, OC, 128], f32,
                                               tag="psum_hw")
                        for oc in range(OC):
                            for half in range(2):
                                for j in range(HC // 2):
                                    hc = half * (HC // 2) + j
                                    nc.tensor.matmul(
                                        out=psum_hw[:, oc, :],
                                        lhsT=h_tiles[half](j),
                                        rhs=w2_sb[:, hc, oc * 128:(oc + 1) * 128],
                                        start=(half == 0 and j == 0),
                                        stop=(half == 1 and j == HC // 2 - 1))
                        hw_sb = hwsb_pool.tile([128, OUT], cdt, tag="hw_sb")
                        nc.vector.tensor_copy(out=hw_sb[:], in_=psum_hw[:])
                        nc.sync.dma_start(out=hw_loc[bw, :], in_=hw_sb[:])

                # ---------------- all-gather of hw ----------------
                if no_collective:
                    nc.gpsimd.dma_start(out=hw_full[0:NLP, :], in_=hw_loc[:])
                else:
                    nc.gpsimd.collective_compute(
                        "AllGather",
                        mybir.AluOpType.bypass,
                        replica_groups=[list(range(C))],
                        ins=[hw_loc[:].opt()],
                        outs=[hw_full[:].opt()],
                    )
                if debug_out:
                    nc.gpsimd.dma_start(out=dbg_hwfull[:], in_=hw_full[:])

                # ---------------- phase B: layer 2 ----------------
                with (
                    tc.tile_pool(name=f"sb_pool{rep}", bufs=2) as sb_pool,
                    tc.tile_pool(name=f"hwg_pool{rep}", bufs=4) as hwg_pool,
                    tc.tile_pool(name=f"hblk_pool{rep}", bufs=2) as hblk_pool,
                    tc.tile_pool(name=f"osb_pool{rep}", bufs=3) as osb_pool,
                    tc.tile_pool(name=f"o_psum{rep}", bufs=3,
                                 space="PSUM") as o_psum,
                ):
                    for b in range(B):
                        bw = slice(b * 128, (b + 1) * 128)
                        if resident:
                            s2_lhs = lambda lo, hi, b=b: s_all_sb[:, b, lo:hi]
                            hb_lhs = lambda k, bw=bw: hT_all[:, k, bw]
                        else:
                            s_tile2 = sb_pool.tile([128, KT * 128], cdt,
                                                   tag="s_sb2")
                            nc.sync.dma_start(out=s_tile2[:],
                                              in_=s_in[:, b, :])
                            s2_lhs = lambda lo, hi, t=s_tile2: t[:, lo:hi]
                            hblk_t = hblk_pool.tile([128, HC, 128], cdt,
                                                    tag="hblk")
                            nc.sync.dma_start(out=hblk_t[:],
                                              in_=hT_d[:, :, bw])
                            hb_lhs = lambda k, t=hblk_t: t[:, k, :]
                        hwg = hwg_pool.tile([128, KT, OUT], cdt, tag="hwg")
                        _gather(hwg, hw_full, idx_sb, idx16_sb, b, OUT)

                        psum_o = o_psum.tile([128, OC, 128], f32, tag="psum_o")
                        for oc in range(OC):
                            ow = slice(oc * 128, (oc + 1) * 128)
                            for kt in range(KT):
                                nc.tensor.matmul(
                                    out=psum_o[:, oc, :],
                                    lhsT=s2_lhs(kt * 128, (kt + 1) * 128),
                                    rhs=hwg[:, kt, ow],
                                    start=(kt == 0), stop=False)
                            for k in range(HC):
                                nc.tensor.matmul(
                                    out=psum_o[:, oc, :],
                                    lhsT=hb_lhs(k),
                                    rhs=rw2b_sb[:, k, ow],
                                    start=False, stop=False)
                            nc.tensor.matmul(
                                out=psum_o[:, oc, :],
                                lhsT=ones_sb[:],
                                rhs=rw2b_sb[:, HC, ow],
                                start=False, stop=True)
                        out_sb = osb_pool.tile([128, OUT], f32, tag="out_sb")
                        nc.vector.tensor_copy(out=out_sb[:], in_=psum_o[:])
                        nc.sync.dma_start(out=out_d[bw, :], in_=out_sb[:])

    nc.compile()
    return nc


# ----------------------------------------------------------------------------
# entry points
# ----------------------------------------------------------------------------
def _run(inputs, trace=False, compute=None, trace_kwargs=None):
    compute = compute or COMPUTE
    cdt, np_cdt = _DT[compute]
    x = np.asarray(inputs["x"])
    cfg = _cfg_from_shapes(x, np.asarray(inputs["w1"]),
                           np.asarray(inputs["w2"]))
    in_maps, KT = _preprocess(
        x, inputs["edge_index"], inputs["edge_weight"],
        inputs["w1"], inputs["b1"], inputs["w2"], inputs["b2"],
        inputs["rw1"], inputs["rb1"], inputs["rw2"], inputs["rb2"],
        cfg, np_cdt)

    key = (tuple(sorted(cfg.items())), KT, compute)
    nc = _prog_cache.get(key)
    if nc is None:
        nc = _build(cfg, KT, cdt)
        _prog_cache[key] = nc

    res = run_bass_kernel_spmd(
        nc, in_maps, core_ids=list(range(C)), trace=trace,
        **(trace_kwargs or {}))

    NL, NLP = cfg["NL"], cfg["NLP"]
    out = np.concatenate(
        [res.results[c]["out"][:NL] for c in range(C)], axis=0)
    return np.ascontiguousarray(out.astype(np.float32)), res


def kernel(**inputs) -> np.ndarray:
    out, _ = _run(inputs, trace=False)
    return out
